# revision 25
# baseline (speedup 1.0000x reference)
# Deformable-conv (DCNv2-style, scrambled-reshape variant) Trainium2 Bass kernel.
# Data-parallel over batch: 8 samples -> 8 NeuronCores.
#
# Optimized pipeline (cost-model-driven rewrite of the working baseline):
#   1. Offset (18ch) + modulation (9ch) convs merged into ONE fp16 matmul
#      per (tile, tap): lhsT [C, 41] with offsets at cols 0:18 and mod at
#      cols 32:41 (base-32 so downstream matmul lhsT slices hit the legal
#      {0,32,64} partition bases).  fp16 matmuls are 1 cyc/row vs fp32's 4.
#   2. PE-transpose conv outputs to pixel-major.  The modulation transpose
#      reads strided columns (col = y*64 + 2k + xi) so a conv over the
#      NORMAL image yields the transposed-pixel order pi2' directly --
#      no second conv over the transposed image.
#   3. Per-n2 selection matmuls (host-constant permutations) -> positions.
#   4. Metadata batched across all 9 n2: one DVE op per stage on
#      [128, 9*KCH*2] instead of 9 small chains.  Scales stored fp16 in
#      duplicated-pair layout [128, 9, KCH, 4, 2].
#   5. Gathers: ONE batched indirect DMA per (sp, n2) (offset AP [128,16],
#      2048 descriptors) instead of 16 calls -- SWDGE fixed overhead is
#      ~1us per call.  Patch table rows are interleaved [ch_hi, corner,
#      ch_lo(2)] so the scale multiply has a stride-1 innermost dim on
#      every operand -> DVE 2x mode.
#   6. Corner combine fused into PE: per 128-pixel chunk, 4 accumulated
#      matmuls (lhsT = scaled corner slice, rhs = identity) transpose and
#      sum the corners in PSUM.  ACT copies PSUM -> fp16 vc.
#   7. Main conv = 9 accumulated fp16 matmuls per [128, 512] output tile;
#      ACT epilogue writes through a transposed AP to undo pi2' ordering;
#      fp16 DMA out (host casts to f32).
import sys

import numpy as np

sys.path.insert(0, "/opt/trn_rl_repo")

import concourse.bass as bass
import concourse.bacc as bacc
import concourse.mybir as mybir
from concourse import tile
from concourse.bass_utils import run_bass_kernel_spmd

F32 = mybir.dt.float32
F16 = mybir.dt.float16
I32 = mybir.dt.int32
I16 = mybir.dt.int16

B, C, H, W = 8, 128, 64, 64
OUT = 256
PIX = H * W            # 4096
KCH = 32               # pixel-major chunks (4096 / 128)
TROWS = 4224           # patch table rows (4096 + pad for f+65 reads)

# f16 const blob layout (per-partition element offsets)
N_WOMT = 9 * 41        # 369
N_SELT = 9 * 3 * 128   # 3456
N_W2 = 9 * 2 * 128     # 2304
N_ID = 128
C16_N = N_WOMT + N_SELT + N_W2 + N_ID
O_WOMT, O_SELT = 0, N_WOMT
O_W2 = O_SELT + N_SELT
O_ID = O_W2 + N_W2
# f32 const blob: byx [9*KCH*2] + bias [1]
C32_N = 9 * KCH * 2 + 1
O_BYX, O_BIAS = 0, 9 * KCH * 2

_CACHE = {}


def _build_host_constants():
    if "sel" in _CACHE:
        return _CACHE
    p2 = np.arange(128)
    k2 = np.arange(KCH)
    sel = np.zeros((9, 3, 128, 128), np.float16)   # [n2, r, p_src, p2]
    byx = np.zeros((128, 9, KCH, 2), np.float32)
    for n2 in range(9):
        a2, e2 = n2 // 3, n2 % 3
        i2 = p2 % 64
        r = (i2 + e2) % 3
        n = 3 * r + a2                       # source kernel point per partition
        J = (64 * e2 + i2) // 3              # source col j per partition
        c_src = 64 * (p2 // 64) + J          # source partition in pixel-major
        for rr in range(3):
            m = r == rr
            sel[n2, rr, c_src[m], p2[m]] = 1.0
        a = n // 3
        e = n % 3
        byx[:, n2, :, 0] = (2 * k2[None, :] + (p2 // 64)[:, None]) + a[:, None]
        byx[:, n2, :, 1] = (J + e)[:, None] * np.ones((1, KCH), np.float32)
    _CACHE["sel"] = sel
    _CACHE["byx"] = byx
    _CACHE["ident16"] = np.eye(128, dtype=np.float16)
    return _CACHE


def _pad66_f16(img):  # [C,64,64] -> [C, 66*66] zero-padded fp16
    p = np.zeros((C, 66, 66), np.float16)
    p[:, 1:65, 1:65] = img
    return p.reshape(C, 66 * 66)


def _patch_table(img):
    # [C,64,64] f32 -> [TROWS, 512] fp16, row = [corner(4), ch(128)]
    flat = np.zeros((C, TROWS + 65), np.float16)
    flat[:, :PIX] = img.reshape(C, PIX).astype(np.float16)
    f = np.arange(TROWS)
    tab = np.stack(
        [flat[:, f], flat[:, f + 1], flat[:, f + 64], flat[:, f + 65]], axis=1
    )  # [C, 4, TROWS]
    return np.ascontiguousarray(tab.transpose(2, 1, 0)).reshape(TROWS, 512)


def _build_program():
    if "nc" in _CACHE:
        return _CACHE["nc"]
    nc = bacc.Bacc()
    d = {}
    d["xpad"] = nc.dram_tensor("xpad", [C, 66 * 66], F16, kind="ExternalInput")
    d["ptab"] = nc.dram_tensor("ptab", [TROWS, 512], F16, kind="ExternalInput")
    d["c16"] = nc.dram_tensor("c16", [128, C16_N], F16, kind="ExternalInput")
    d["c32"] = nc.dram_tensor("c32", [128, C32_N], F32, kind="ExternalInput")
    d["out"] = nc.dram_tensor("out", [OUT, PIX], F16, kind="ExternalOutput")
    d["fidx"] = nc.dram_tensor("fidx", [128, 9 * KCH], I16, kind="Internal")
    DBG = bool(_CACHE.get("debug"))
    if DBG:
        d["dbg_cm"] = nc.dram_tensor("dbg_cm", [128, PIX], F16, kind="ExternalOutput")
        d["dbg_tmpm"] = nc.dram_tensor("dbg_tmpm", [128, PIX], F16, kind="ExternalOutput")
        d["dbg_opm"] = nc.dram_tensor("dbg_opm", [128, KCH * 18], F16, kind="ExternalOutput")
        d["dbg_mpm"] = nc.dram_tensor("dbg_mpm", [128, 9 * KCH], F16, kind="ExternalOutput")
        d["dbg_P"] = nc.dram_tensor("dbg_P", [128, 9 * KCH * 2], F32, kind="ExternalOutput")
        d["dbg_idx"] = nc.dram_tensor("dbg_idx", [128, 9 * KCH], I16, kind="ExternalOutput")
        d["dbg_scal"] = nc.dram_tensor("dbg_scal", [128, 9 * KCH * 8], F16, kind="ExternalOutput")
        d["dbg_g"] = nc.dram_tensor("dbg_g", [128, 16 * 512], F16, kind="ExternalOutput")
        d["dbg_vc"] = nc.dram_tensor("dbg_vc", [128, 9 * 2048], F16, kind="ExternalOutput")

    AO = mybir.AluOpType
    AF = mybir.ActivationFunctionType

    with tile.TileContext(nc) as tc:
        with (
            tc.tile_pool(name="imgs", bufs=1) as imgs,
            tc.tile_pool(name="wts", bufs=1) as wts,
            tc.tile_pool(name="meta", bufs=1) as meta,
            tc.tile_pool(name="ps", bufs=6, space="PSUM") as psp,
            tc.tile_pool(name="psmm", bufs=2, space="PSUM") as psm,
            tc.tile_pool(name="gbuf", bufs=3) as gbuf,
            tc.tile_pool(name="vbuf", bufs=2) as vbuf,
            tc.tile_pool(name="obuf", bufs=2) as obuf,
        ):
            # ---- bulk loads: image + two constant blobs
            xpad = imgs.tile([C, 66 * 66], F16)
            c16 = wts.tile([128, C16_N], F16)
            c32 = wts.tile([128, C32_N], F32)
            nc.sync.dma_start(xpad[:], d["xpad"][:])
            nc.sync.dma_start(c16[:], d["c16"][:])
            nc.sync.dma_start(c32[:], d["c32"][:])

            def c16ap(off, dims):
                return bass.AP(tensor=c16[:].tensor, offset=c16[:].offset + off,
                               ap=[list(c16[:].ap[0])] + dims)

            def c32ap(off, dims):
                return bass.AP(tensor=c32[:].tensor, offset=c32[:].offset + off,
                               ap=[list(c32[:].ap[0])] + dims)

            id16 = c16ap(O_ID, [[1, 128]])

            # ---- merged conv (41ch out: 0:18 offsets, 32:41 sigmoid-mod).
            # The sigmoid epilogue writes tmpm in pi2' (= x*64+y) column
            # order directly via a strided dst AP.
            cm = meta.tile([128, PIX], F16)    # partitions 0:18 offsets (pi)
            tmpm = meta.tile([128, PIX], F16)  # partitions 0:9 mod (pi2')
            for tl in range(8):
                po = psp.tile([128, 512], F32, tag="ps")
                for t in range(9):
                    dy, dx = t // 3, t % 3
                    off = dy * 66 + dx + tl * 8 * 66
                    rhs = bass.AP(
                        tensor=xpad[:].tensor, offset=xpad[:].offset + off,
                        ap=[list(xpad[:].ap[0]), [66, 8], [1, 64]],
                    )
                    nc.tensor.matmul(po[0:41, :], c16ap(O_WOMT + t * 41, [[1, 41]]),
                                     rhs, start=(t == 0), stop=(t == 8))
                cs = slice(tl * 512, (tl + 1) * 512)
                nc.scalar.activation(cm[0:18, cs], po[0:18, :], AF.Identity,
                                     bias=c32[0:18, O_BIAS:O_BIAS + 1],
                                     scale=1.0)
                dst_s = bass.AP(tensor=tmpm[32:41, 0:1].tensor,
                                offset=tmpm[32:41, 0:1].offset + 8 * tl,
                                ap=[list(tmpm[32:41, 0:1].ap[0]), [1, 8], [64, 64]])
                nc.scalar.activation(dst_s, po[32:41, :], AF.Sigmoid,
                                     bias=c32[32:41, O_BIAS:O_BIAS + 1],
                                     scale=1.0)

            # ---- PE-transpose conv outputs to pixel-major (4 chunks / bank)
            opm = meta.tile([128, KCH, 18], F16)   # pi = 128k+p
            mpm = meta.tile([128, 9, KCH], F16)    # pi2' = 128k+p, n2-major
            id_o = bass.AP(tensor=c16[:].tensor,
                           offset=c16[0:18, 0:1].offset + O_ID,
                           ap=[list(c16[0:18, 0:1].ap[0]), [1, 128]])
            id_m = bass.AP(tensor=c16[:].tensor,
                           offset=c16[32:41, 0:1].offset + O_ID,
                           ap=[list(c16[32:41, 0:1].ap[0]), [1, 128]])
            for kb in range(8):
                pt = psp.tile([128, 512], F32, tag="ps")
                pm = psp.tile([128, 512], F32, tag="ps")
                for j in range(4):
                    k = kb * 4 + j
                    nc.tensor.matmul(pt[:, j * 128:(j + 1) * 128],
                                     cm[0:18, k * 128:(k + 1) * 128],
                                     id_o, start=True, stop=True)
                    nc.tensor.matmul(pm[:, j * 128:(j + 1) * 128],
                                     tmpm[32:41, k * 128:(k + 1) * 128],
                                     id_m, start=True, stop=True)
                # copies: offsets [128, 4, 18] ; mod [128, 9, 4] (n2-major dst)
                src_o = bass.AP(tensor=pt[:].tensor, offset=pt[:].offset,
                                ap=[list(pt[:].ap[0]), [128, 4], [1, 18]])
                nc.vector.tensor_copy(opm[:, kb * 4:(kb + 1) * 4, :], src_o)
                src_m = bass.AP(tensor=pm[:].tensor, offset=pm[:].offset + 32,
                                ap=[list(pm[:].ap[0]), [1, 9], [128, 4]])
                dst_m = bass.AP(tensor=mpm[:].tensor,
                                offset=mpm[:].offset + kb * 4,
                                ap=[list(mpm[:].ap[0]), [KCH, 9], [1, 4]])
                nc.vector.tensor_copy(dst_m, src_m)

            # ---- selection matmuls -> positions; metadata in 3 n2-groups so
            # the first gathers can start while later groups still compute.
            P = meta.tile([128, 9, KCH, 2], F32)
            R0 = meta.tile([128, 9, KCH, 2], F32)
            F = meta.tile([128, 9, KCH, 2], F32)
            f00 = meta.tile([128, 9, KCH], F32)
            idxt = meta.tile([128, 9, KCH], I16)
            idx16 = meta.tile([128, 9, KCH, 8], I16)  # 16-part wrap, 8x repl
            scal = meta.tile([128, 9, KCH, 4, 2], F16)
            v1 = meta.tile([128, 9, KCH], F32)
            v0 = meta.tile([128, 9, KCH], F32)
            for grp in range(3):
                gs = slice(3 * grp, 3 * grp + 3)
                for n2 in range(3 * grp, 3 * grp + 3):
                    oyx = psp.tile([128, KCH, 2], F32, tag="ps")
                    for r in range(3):
                        a2 = n2 // 3
                        ch = 3 * r + a2
                        rhs = bass.AP(
                            tensor=opm[:].tensor, offset=opm[:].offset + ch,
                            ap=[list(opm[:].ap[0]), [18, KCH], [9, 2]],
                        )
                        nc.tensor.matmul(oyx[:],
                                         c16ap(O_SELT + (n2 * 3 + r) * 128,
                                               [[1, 128]]),
                                         rhs, start=(r == 0), stop=(r == 2))
                    nc.vector.tensor_add(P[:, n2], oyx[:],
                                         c32ap(O_BYX + n2 * KCH * 2,
                                               [[1, KCH * 2]]))
                nc.vector.tensor_scalar(P[:, gs], P[:, gs], 0.0, 63.0,
                                        AO.max, AO.min)
                nc.vector.tensor_scalar(R0[:, gs], P[:, gs], -0.5, 12582912.0,
                                        AO.add, AO.add)
                nc.vector.tensor_scalar_add(R0[:, gs], R0[:, gs], -12582912.0)
                nc.vector.tensor_sub(F[:, gs], P[:, gs], R0[:, gs])
                nc.vector.scalar_tensor_tensor(
                    f00[:, gs], R0[:, gs, :, 1], 64.0, R0[:, gs, :, 0],
                    AO.mult, AO.add)
                nc.vector.tensor_copy(idxt[:, gs], f00[:, gs])
                # dma_gather wants idxs wrapped into 16 partitions and
                # replicated across the 8 Q7 cores: round-trip through DRAM.
                # fidx layout: [r(16), grp(3), nk(96), s(8)] so the store
                # splits the partition dim p = 16s + r on the DRAM side and
                # each core-block load is a contiguous run.
                for s in range(8):
                    sblk = idxt[16 * s:16 * (s + 1), 0, 0:1]
                    st_src = bass.AP(tensor=sblk.tensor,
                                     offset=sblk.offset + 96 * grp,
                                     ap=[list(sblk.ap[0]), [1, 96]])
                    st_dst = bass.AP(tensor=d["fidx"][:].tensor,
                                     offset=grp * 768 + s,
                                     ap=[[2304, 16], [8, 96]])
                    nc.sync.dma_start(st_dst, st_src)
                for c in range(8):
                    blk = idx16[16 * c:16 * (c + 1), 0, 0, 0:1]
                    ld_dst = bass.AP(
                        tensor=blk.tensor,
                        offset=blk.offset + grp * 768,
                        ap=[list(blk.ap[0]), [1, 768]])
                    ld_src = bass.AP(
                        tensor=d["fidx"][:].tensor, offset=grp * 768,
                        ap=[[2304, 16], [1, 768]])
                    nc.sync.dma_start(ld_dst, ld_src)
                # scales fp16, duplicated pairs: scal[p, n2, kk, corner, 2]
                nc.vector.tensor_mul(v1[:, gs], mpm[:, gs], F[:, gs, :, 1])
                nc.vector.tensor_sub(v0[:, gs], mpm[:, gs], v1[:, gs])
                for l in range(2):
                    nc.vector.tensor_mul(scal[:, gs, :, 1, l], v0[:, gs],
                                         F[:, gs, :, 0])
                    nc.vector.tensor_mul(scal[:, gs, :, 3, l], v1[:, gs],
                                         F[:, gs, :, 0])
                for l in range(2):
                    nc.vector.tensor_sub(scal[:, gs, :, 0, l], v0[:, gs],
                                         scal[:, gs, :, 1, 0])
                    nc.vector.tensor_sub(scal[:, gs, :, 2, l], v1[:, gs],
                                         scal[:, gs, :, 3, 0])

            if DBG:
                nc.sync.dma_start(d["dbg_cm"][:], cm[:])
                nc.sync.dma_start(d["dbg_tmpm"][:], tmpm[:])
                nc.sync.dma_start(
                    d["dbg_opm"][:],
                    bass.AP(tensor=opm[:].tensor, offset=opm[:].offset,
                            ap=[list(opm[:].ap[0]), [1, KCH * 18]]))
                nc.sync.dma_start(
                    d["dbg_mpm"][:],
                    bass.AP(tensor=mpm[:].tensor, offset=mpm[:].offset,
                            ap=[list(mpm[:].ap[0]), [1, 9 * KCH]]))
                nc.sync.dma_start(
                    d["dbg_P"][:],
                    bass.AP(tensor=P[:].tensor, offset=P[:].offset,
                            ap=[list(P[:].ap[0]), [1, 9 * KCH * 2]]))
                nc.sync.dma_start(
                    d["dbg_idx"][:],
                    bass.AP(tensor=idxt[:].tensor, offset=idxt[:].offset,
                            ap=[list(idxt[:].ap[0]), [1, 9 * KCH]]))
                nc.sync.dma_start(
                    d["dbg_scal"][:],
                    bass.AP(tensor=scal[:].tensor, offset=scal[:].offset,
                            ap=[list(scal[:].ap[0]), [1, 9 * KCH * 8]]))

            # ---- per (sp, n2): batched gather + scale + PE corner-transpose
            for sp in range(2):
                vc = vbuf.tile([C, 9, 16 * 128], F16, tag="vc")
                for n2 in range(9):
                    g = gbuf.tile([128, 16, 4, 128], F16, tag="g")
                    # 1024 idxs per call -- 2048 overflows the SWDGE ring
                    for h in range(2):
                        gdst = bass.AP(
                            tensor=g[:].tensor,
                            offset=g[:].offset + h * 8 * 512,
                            ap=[list(g[:].ap[0]), [512, 8], [1, 512]])
                        idxs = bass.AP(
                            tensor=idx16[:].tensor,
                            offset=idx16[:].offset
                            + (n2 * KCH + sp * 16 + h * 8) * 8,
                            ap=[list(idx16[:].ap[0]), [8, 8], [1, 8]])
                        nc.gpsimd.dma_gather(
                            gdst, d["ptab"][:], idxs, 8 * 128, 8 * 128, 512)
                    # scale: g[p,kk,c,ch] *= scal[p,n2,kk,c] -- the dup-pair
                    # scal layout pairs the contiguous ch dim so every AP has
                    # a stride-1 innermost dim (DVE 2x mode).
                    sc = bass.AP(
                        tensor=scal[:].tensor,
                        offset=scal[:].offset + (n2 * KCH + sp * 16) * 8,
                        ap=[list(scal[:].ap[0]), [8, 16], [2, 4], [0, 64], [1, 2]],
                    )
                    nc.vector.tensor_mul(g[:], g[:], sc)
                    # 4 chunks per PSUM bank; 4 corner-accumulating transposes
                    for qb in range(4):
                        ptv = psp.tile([128, 512], F32, tag="ps")
                        for j in range(4):
                            kk = qb * 4 + j
                            for cr in range(4):
                                lhs = bass.AP(
                                    tensor=g[:].tensor,
                                    offset=g[:].offset + kk * 512 + cr * 128,
                                    ap=[list(g[:].ap[0]), [1, 128]],
                                )
                                nc.tensor.matmul(
                                    ptv[:, j * 128:(j + 1) * 128], lhs, id16,
                                    start=(cr == 0), stop=(cr == 3))
                        nc.scalar.copy(
                            vc[:, n2, qb * 512:(qb + 1) * 512], ptv[:])
                    if DBG and sp == 0 and n2 == 0:
                        gd = bass.AP(tensor=g[:].tensor, offset=g[:].offset,
                                     ap=[list(g[:].ap[0]), [1, 16 * 512]])
                        nc.sync.dma_start(d["dbg_g"][:], gd)
                if DBG and sp == 0:
                    vd = bass.AP(tensor=vc[:].tensor, offset=vc[:].offset,
                                 ap=[list(vc[:].ap[0]), [1, 9 * 2048]])
                    nc.sync.dma_start(d["dbg_vc"][:], vd)

                # main conv on this spatial half (pi2' in [sp*2048, +2048))
                for hf in range(2):
                    outsb = obuf.tile([128, 16 * 128], F16, tag="osb")
                    for tl in range(4):
                        acc = psm.tile([128, 512], F32, tag="mm")
                        for n2 in range(9):
                            nc.tensor.matmul(
                                acc[:],
                                c16ap(O_W2 + (n2 * 2 + hf) * 128, [[1, 128]]),
                                vc[:, n2, tl * 512:(tl + 1) * 512],
                                start=(n2 == 0), stop=(n2 == 8))
                        # acc covers pi2' = sp*2048 + tl*512 + [0,512):
                        # j2 = (pi2'//64), i2 = pi2'%64 -> dst elem i2*32 + (j2 - 32*sp)
                        dstap = bass.AP(
                            tensor=outsb[:].tensor,
                            offset=outsb[:].offset + 8 * tl,
                            ap=[list(outsb[:].ap[0]), [1, 8], [32, 64]],
                        )
                        nc.scalar.copy(dstap, acc[:])
                    # DMA: out[128hf + o, i2, 32sp + j2'] <- outsb[o, i2*32 + j2']
                    dd = d["out"]
                    dram = bass.AP(
                        tensor=dd[:].tensor,
                        offset=dd[:].offset + hf * 128 * PIX + 32 * sp,
                        ap=[[PIX, 128], [64, 64], [1, 32]],
                    )
                    nc.sync.dma_start(dram, outsb[:])

    nc.compile()
    _CACHE["nc"] = nc
    return nc


def _host_inputs(b_x, offset_w, offset_b, mod_w, mod_b, conv_w):
    hc = _build_host_constants()
    img = b_x.astype(np.float32)
    if "c16_static" not in _CACHE:
        womt = np.zeros((9, C, 41), np.float16)
        for t in range(9):
            dy, dx = t // 3, t % 3
            womt[t, :, 0:18] = offset_w[:, :, dy, dx].T.astype(np.float16)
            womt[t, :, 32:41] = mod_w[:, :, dy, dx].T.astype(np.float16)
        w2 = np.zeros((9, 2, C, 128), np.float16)
        for n2 in range(9):
            a2, e2 = n2 // 3, n2 % 3
            for hf in range(2):
                w2[n2, hf] = conv_w[128 * hf:128 * (hf + 1), :, a2, e2].T.astype(
                    np.float16)
        c16 = np.zeros((128, C16_N), np.float16)
        c16[:, O_WOMT:O_WOMT + N_WOMT] = womt.transpose(1, 0, 2).reshape(C, N_WOMT)
        c16[:, O_SELT:O_SELT + N_SELT] = (
            hc["sel"].transpose(2, 0, 1, 3).reshape(128, N_SELT))
        c16[:, O_W2:O_W2 + N_W2] = w2.transpose(2, 0, 1, 3).reshape(C, N_W2)
        c16[:, O_ID:O_ID + N_ID] = hc["ident16"]
        c32 = np.zeros((128, C32_N), np.float32)
        c32[:, O_BYX:O_BYX + 9 * KCH * 2] = hc["byx"].reshape(128, 9 * KCH * 2)
        bias = np.zeros((128, 1), np.float32)
        bias[0:18, 0] = offset_b.astype(np.float32)
        bias[32:41, 0] = mod_b.astype(np.float32)
        c32[:, O_BIAS:O_BIAS + 1] = bias
        _CACHE["c16_static"] = c16
        _CACHE["c32_static"] = c32
    return {
        "xpad": _pad66_f16(img),
        "ptab": _patch_table(img),
        "c16": _CACHE["c16_static"],
        "c32": _CACHE["c32_static"],
    }


def kernel(x, offset_w, offset_b, mod_w, mod_b, conv_w):
    nc = _build_program()
    _CACHE.pop("c16_static", None)
    _CACHE.pop("c32_static", None)
    in_maps = [
        _host_inputs(x[b], offset_w, offset_b, mod_w, mod_b, conv_w)
        for b in range(B)
    ]
    res = run_bass_kernel_spmd(nc, in_maps, core_ids=list(range(B)))
    out = np.stack([res.results[b]["out"].reshape(OUT, H, W) for b in range(B)])
    return out.astype(np.float32)


if __name__ == "__main__":
    rng = np.random.default_rng(0)
    ins = {
        "x": rng.standard_normal((B, C, H, W), dtype=np.float32),
        "offset_w": (rng.standard_normal((18, C, 3, 3)) / 34).astype(np.float32),
        "offset_b": (rng.standard_normal(18) * 0.01).astype(np.float32),
        "mod_w": (rng.standard_normal((9, C, 3, 3)) / 34).astype(np.float32),
        "mod_b": (rng.standard_normal(9) * 0.01).astype(np.float32),
        "conv_w": (rng.standard_normal((OUT, C, 3, 3)) / 34).astype(np.float32),
    }
    o = kernel(**ins)
    print("out", o.shape, o.dtype, np.abs(o).max())
